# revision 7
# baseline (speedup 1.0000x reference)
"""LCA layer kernel for Trainium2, data-parallel over tokens on 8 NeuronCores.

Reference computation (per token row x of d_model=1024, W [1024, 4096]):
    b = x @ W;  G = W^T W with zero diag;  u_0 = 0
    10x: a = relu(u - 0.1); u = 0.9 u + 0.1 (b - a @ G)
    out = relu(u - 0.1) @ W^T

Device algorithm (per core, 1024 tokens, everything in SBUF):
  * factor a@G = (a @ W^T) @ W - g * a   with g = diag(W^T W), halving FLOPs
    and avoiding the 64 MB G matrix entirely.
  * fold the dt/tau=0.1 into W01 = 0.1*W, so B' = x@W01 = 0.1 b,
    Y' = (a@W^T)@W01 = 0.1 a W^T W,  and g2 = 0.1*g.
  * u after step 1 is exactly B' (u0=0 -> a0=0), so only 9 iterated steps.
  * update: u' = 0.9 u + B' - Y' + relu(g2*u - 0.1*g2)   [g2*relu(u-0.1)
    equals relu(g2*u - 0.1*g2) since g2 > 0 -- computed on the ACT engine
    with per-partition scale/bias].
  * all state is stored transposed [feature, token] so the per-partition
    quantities (g2) line up with the partition dim and no transposes are
    needed anywhere in the iteration.
  * matmuls in bf16 (fp32 PSUM accumulation); u/B'-init kept fp32.
    Measured emulation error vs fp32 reference: rel_l2 ~ 2.7e-3.
"""

import numpy as np
import ml_dtypes

P = 128          # partitions
T = 256          # tokens per block
NBLK = 4         # blocks per core (4*256 = 1024 tokens/core)
NSTEPS = 9       # iterated steps (step 1 is the free u=B' init)
DM = 1024        # d_model
DL = 4096        # d_lca
NDM = DM // P    # 8 d_model chunks
NDL = DL // P    # 32 d_lca chunks
NCORES = 8
TOK_CORE = NBLK * T

BF16 = ml_dtypes.bfloat16

_CACHE = {}

# Set TRACE=True (e.g. from a test harness) to request an NTFF profile;
# the BassKernelResults lands in LAST_RESULT either way.
TRACE = False
LAST_RESULT = None


def _build_nc():
    import concourse.bacc as bacc
    import concourse.tile as tile
    import concourse.mybir as mybir

    dt = mybir.dt
    Alu = mybir.AluOpType
    Act = mybir.ActivationFunctionType

    nc = bacc.Bacc("TRN2", target_bir_lowering=False, debug=False,
                   num_devices=NCORES)

    xt_d = nc.dram_tensor("xt", [NBLK, P, NDM, T], dt.bfloat16,
                          kind="ExternalInput").ap()
    wt_d = nc.dram_tensor("wt", [P, NDL, DM], dt.bfloat16,
                          kind="ExternalInput").ap()
    w01_d = nc.dram_tensor("w01", [P, NDM, DL], dt.bfloat16,
                           kind="ExternalInput").ap()
    g2_d = nc.dram_tensor("g2", [P, NDL], dt.float32,
                          kind="ExternalInput").ap()
    gb2_d = nc.dram_tensor("gb2", [P, NDL], dt.float32,
                           kind="ExternalInput").ap()
    out_d = nc.dram_tensor("out", [TOK_CORE, DM], dt.float32,
                           kind="ExternalOutput").ap()

    with tile.TileContext(nc) as tc:
        with (
            tc.tile_pool(name="wpool", bufs=1) as wpool,
            tc.tile_pool(name="state", bufs=1) as state,
            tc.tile_pool(name="htp", bufs=1) as htp,
            tc.tile_pool(name="xio", bufs=1) as xio,
            tc.tile_pool(name="oio", bufs=1) as oio,
            tc.tile_pool(name="tmp", bufs=3) as tmp,
            tc.tile_pool(name="psum", bufs=8, space="PSUM") as psum,
        ):
            # ---- resident weights ----
            wt = wpool.tile([P, NDL, DM], dt.bfloat16, tag="wt")
            w01 = wpool.tile([P, NDM, DL], dt.bfloat16, tag="w01")
            g2 = wpool.tile([P, NDL], dt.float32, tag="g2")
            gb2 = wpool.tile([P, NDL], dt.float32, tag="gb2")
            nlam = wpool.tile([P, 1], dt.float32, tag="nlam")
            nc.gpsimd.memset(nlam[:], -0.1)
            for dmc in range(NDM):
                nc.sync.dma_start(w01[:, dmc, :], w01_d[:, dmc, :])
            for kg in range(0, NDL, 8):
                nc.sync.dma_start(wt[:, kg:kg + 8, :], wt_d[:, kg:kg + 8, :])
            nc.sync.dma_start(g2[:], g2_d[:])
            nc.sync.dma_start(gb2[:], gb2_d[:])

            for blk in range(NBLK):
                xt = xio.tile([P, NDM, T], dt.bfloat16, tag="xt")
                nc.sync.dma_start(xt[:], xt_d[blk])

                u = state.tile([P, NDL, T], dt.float32, tag="u")
                bp = state.tile([P, NDL, T], dt.bfloat16, tag="bp")
                a = state.tile([P, NDL, T], dt.bfloat16, tag="a")

                # ---- B' = x @ W01 (transposed out), u <- B' ----
                for jc in range(NDL):
                    pb = psum.tile([P, T], dt.float32, tag="mm")
                    for dmc in range(NDM):
                        nc.tensor.matmul(
                            pb[:], w01[:, dmc, jc * P:(jc + 1) * P],
                            xt[:, dmc, :],
                            start=(dmc == 0), stop=(dmc == NDM - 1))
                    nc.scalar.copy(u[:, jc, :], pb[:])
                    nc.vector.tensor_copy(bp[:, jc, :], pb[:])
                    # a for the first iterated step
                    nc.scalar.activation(a[:, jc, :], u[:, jc, :], Act.Relu,
                                         bias=nlam[:, 0:1])

                # ---- 9 iterated steps ----
                for _ in range(NSTEPS):
                    # hT = (a @ W^T)^T = W a^T, bf16 [dm, tok]
                    ht = htp.tile([P, NDM, T], dt.bfloat16, tag="ht")
                    for dmc in range(NDM):
                        ph = psum.tile([P, T], dt.float32, tag="mm")
                        for kc in range(NDL):
                            nc.tensor.matmul(
                                ph[:], wt[:, kc, dmc * P:(dmc + 1) * P],
                                a[:, kc, :],
                                start=(kc == 0), stop=(kc == NDL - 1))
                        nc.scalar.copy(ht[:, dmc, :], ph[:])

                    # Y' per chunk, then fused update of u and next a
                    for jc in range(NDL):
                        py = psum.tile([P, T], dt.float32, tag="mm")
                        for dmc in range(NDM):
                            nc.tensor.matmul(
                                py[:], w01[:, dmc, jc * P:(jc + 1) * P],
                                ht[:, dmc, :],
                                start=(dmc == 0), stop=(dmc == NDM - 1))
                        # GA = relu(g2*u - 0.1*g2) = g2 * relu(u - 0.1)
                        ga = tmp.tile([P, T], dt.float32, tag="tmp")
                        nc.scalar.activation(ga[:], u[:, jc, :], Act.Relu,
                                             bias=gb2[:, jc:jc + 1],
                                             scale=g2[:, jc:jc + 1])
                        # s2 = GA - Y'  (written back into the PSUM tile)
                        nc.vector.scalar_tensor_tensor(
                            py[:], py[:], -1.0, ga[:],
                            op0=Alu.mult, op1=Alu.add)
                        # s3 = 0.9*u + s2
                        s3 = tmp.tile([P, T], dt.float32, tag="tmp")
                        nc.vector.scalar_tensor_tensor(
                            s3[:], u[:, jc, :], 0.9, py[:],
                            op0=Alu.mult, op1=Alu.add)
                        # u' = s3 + B'
                        nc.vector.tensor_tensor(
                            u[:, jc, :], s3[:], bp[:, jc, :], op=Alu.add)
                        # next a = relu(u' - 0.1)
                        nc.scalar.activation(a[:, jc, :], u[:, jc, :],
                                             Act.Relu, bias=nlam[:, 0:1])

                # ---- out = a @ W^T, natural [tok, dm] layout ----
                for sub in range(T // P):
                    ob = oio.tile([P, DM], dt.float32, tag="ob")
                    for nh in range(2):
                        po = psum.tile([P, 512], dt.float32, tag="mm")
                        for kc in range(NDL):
                            nc.tensor.matmul(
                                po[:], a[:, kc, sub * P:(sub + 1) * P],
                                wt[:, kc, nh * 512:(nh + 1) * 512],
                                start=(kc == 0), stop=(kc == NDL - 1))
                        nc.scalar.copy(ob[:, nh * 512:(nh + 1) * 512], po[:])
                    row = (blk * (T // P) + sub) * P
                    nc.sync.dma_start(out_d[row:row + P, :], ob[:])

    nc.compile()
    return nc


def _get_nc():
    if "nc" not in _CACHE:
        _CACHE["nc"] = _build_nc()
    return _CACHE["nc"]


def _prep_shared(W):
    W = np.asarray(W, np.float32)
    wt = np.ascontiguousarray(
        W.T.reshape(NDL, P, DM).transpose(1, 0, 2)).astype(BF16)
    w01 = np.ascontiguousarray(
        (0.1 * W).reshape(NDM, P, DL).transpose(1, 0, 2)).astype(BF16)
    g = 0.1 * (W.astype(np.float64) ** 2).sum(0).astype(np.float64)
    g2 = np.ascontiguousarray(g.reshape(NDL, P).T).astype(np.float32)
    gb2 = (-0.1 * g2).astype(np.float32)
    return wt, w01, g2, gb2


def kernel(x, W):
    from concourse.bass_utils import run_bass_kernel_spmd

    x = np.asarray(x)
    orig_shape = x.shape
    xf = x.reshape(-1, DM).astype(np.float32)
    wt, w01, g2, gb2 = _prep_shared(W)

    in_maps = []
    for c in range(NCORES):
        xs = xf[c * TOK_CORE:(c + 1) * TOK_CORE]          # [1024, 1024]
        xt = np.ascontiguousarray(
            xs.reshape(NBLK, T, NDM, P).transpose(0, 3, 2, 1)).astype(BF16)
        in_maps.append({"xt": xt, "wt": wt, "w01": w01,
                        "g2": g2, "gb2": gb2})

    nc = _get_nc()
    res = run_bass_kernel_spmd(nc, in_maps, core_ids=list(range(NCORES)),
                               trace=TRACE)
    global LAST_RESULT
    LAST_RESULT = res
    out = np.concatenate([res.results[c]["out"] for c in range(NCORES)], axis=0)
    return out.reshape(orig_shape).astype(np.float32)


# revision 8
# speedup vs baseline: 1.0701x; 1.0701x over previous
"""LCA layer kernel for Trainium2, data-parallel over tokens on 8 NeuronCores.

Reference computation (per token row x of d_model=1024, W [1024, 4096]):
    b = x @ W;  G = W^T W with zero diag;  u_0 = 0
    10x: a = relu(u - 0.1); u = 0.9 u + 0.1 (b - a @ G)
    out = relu(u - 0.1) @ W^T

Device algorithm (per core, 1024 tokens, everything in SBUF):
  * factor a@G = (a @ W^T) @ W - g * a   with g = diag(W^T W), halving FLOPs
    and avoiding the 64 MB G matrix entirely.
  * fold the dt/tau=0.1 into W01 = 0.1*W, so B' = x@W01 = 0.1 b,
    Y' = (a@W^T)@W01 = 0.1 a W^T W,  and g2 = 0.1*g.
  * u after step 1 is exactly B' (u0=0 -> a0=0), so only 9 iterated steps.
  * update: u' = 0.9 u + B' - Y' + relu(g2*u - 0.1*g2)   [g2*relu(u-0.1)
    equals relu(g2*u - 0.1*g2) since g2 > 0 -- computed on the ACT engine
    with per-partition scale/bias].
  * all state is stored transposed [feature, token] so the per-partition
    quantities (g2) line up with the partition dim and no transposes are
    needed anywhere in the iteration.
  * matmuls in bf16 (fp32 PSUM accumulation); u/B'-init kept fp32.
    Measured emulation error vs fp32 reference: rel_l2 ~ 2.7e-3.
"""

import numpy as np
import ml_dtypes

P = 128          # partitions
T = 256          # tokens per block
NBLK = 4         # blocks per core (4*256 = 1024 tokens/core)
NSTEPS = 9       # iterated steps (step 1 is the free u=B' init)
DM = 1024        # d_model
DL = 4096        # d_lca
NDM = DM // P    # 8 d_model chunks
NDL = DL // P    # 32 d_lca chunks
NCORES = 8
TOK_CORE = NBLK * T

BF16 = ml_dtypes.bfloat16

_CACHE = {}

# Set TRACE=True (e.g. from a test harness) to request an NTFF profile;
# the BassKernelResults lands in LAST_RESULT either way.
TRACE = False
LAST_RESULT = None


def _build_nc():
    import concourse.bacc as bacc
    import concourse.tile as tile
    import concourse.mybir as mybir

    dt = mybir.dt
    Alu = mybir.AluOpType
    Act = mybir.ActivationFunctionType

    nc = bacc.Bacc("TRN2", target_bir_lowering=False, debug=False,
                   num_devices=NCORES)

    xt_d = nc.dram_tensor("xt", [NBLK, P, NDM, T], dt.bfloat16,
                          kind="ExternalInput").ap()
    wt_d = nc.dram_tensor("wt", [P, NDL, DM], dt.bfloat16,
                          kind="ExternalInput").ap()
    w01_d = nc.dram_tensor("w01", [P, NDM, DL], dt.bfloat16,
                           kind="ExternalInput").ap()
    g2_d = nc.dram_tensor("g2", [P, NDL], dt.float32,
                          kind="ExternalInput").ap()
    gb2_d = nc.dram_tensor("gb2", [P, NDL], dt.float32,
                           kind="ExternalInput").ap()
    out_d = nc.dram_tensor("out", [TOK_CORE, DM], dt.float32,
                           kind="ExternalOutput").ap()

    with tile.TileContext(nc) as tc:
        with (
            tc.tile_pool(name="wpool", bufs=1) as wpool,
            tc.tile_pool(name="state", bufs=1) as state,
            tc.tile_pool(name="htp", bufs=1) as htp,
            tc.tile_pool(name="xio", bufs=1) as xio,
            tc.tile_pool(name="oio", bufs=1) as oio,
            tc.tile_pool(name="tmp", bufs=3) as tmp,
            tc.tile_pool(name="psum", bufs=8, space="PSUM") as psum,
        ):
            # ---- resident weights ----
            wt = wpool.tile([P, NDL, DM], dt.bfloat16, tag="wt")
            w01 = wpool.tile([P, NDM, DL], dt.bfloat16, tag="w01")
            g2 = wpool.tile([P, NDL], dt.float32, tag="g2")
            gb2 = wpool.tile([P, NDL], dt.float32, tag="gb2")
            nlam = wpool.tile([P, 1], dt.float32, tag="nlam")
            nc.gpsimd.memset(nlam[:], -0.1)
            for dmc in range(NDM):
                nc.sync.dma_start(w01[:, dmc, :], w01_d[:, dmc, :])
            for kg in range(0, NDL, 8):
                nc.sync.dma_start(wt[:, kg:kg + 8, :], wt_d[:, kg:kg + 8, :])
            nc.sync.dma_start(g2[:], g2_d[:])
            nc.sync.dma_start(gb2[:], gb2_d[:])

            for blk in range(NBLK):
                xt = xio.tile([P, NDM, T], dt.bfloat16, tag="xt")
                nc.sync.dma_start(xt[:], xt_d[blk])

                u = state.tile([P, NDL, T], dt.float32, tag="u")
                bp = state.tile([P, NDL, T], dt.bfloat16, tag="bp")
                a = state.tile([P, NDL, T], dt.bfloat16, tag="a")

                # ---- B' = x @ W01 (transposed out), u <- B' ----
                for jc in range(NDL):
                    pb = psum.tile([P, T], dt.float32, tag="mm")
                    for dmc in range(NDM):
                        nc.tensor.matmul(
                            pb[:], w01[:, dmc, jc * P:(jc + 1) * P],
                            xt[:, dmc, :],
                            start=(dmc == 0), stop=(dmc == NDM - 1))
                    nc.scalar.copy(u[:, jc, :], pb[:])
                    nc.vector.tensor_copy(bp[:, jc, :], pb[:])
                    # a for the first iterated step
                    nc.scalar.activation(a[:, jc, :], u[:, jc, :], Act.Relu,
                                         bias=nlam[:, 0:1])

                # ---- 9 iterated steps ----
                for _ in range(NSTEPS):
                    # hT = (a @ W^T)^T = W a^T, bf16 [dm, tok]
                    ht = htp.tile([P, NDM, T], dt.bfloat16, tag="ht")
                    for dmc in range(NDM):
                        ph = psum.tile([P, T], dt.float32, tag="mm")
                        for kc in range(NDL):
                            nc.tensor.matmul(
                                ph[:], wt[:, kc, dmc * P:(dmc + 1) * P],
                                a[:, kc, :],
                                start=(kc == 0), stop=(kc == NDL - 1))
                        nc.scalar.copy(ht[:, dmc, :], ph[:])

                    # Y' per chunk, then fused update of u and next a
                    for jc in range(NDL):
                        py = psum.tile([P, T], dt.float32, tag="mm")
                        for dmc in range(NDM):
                            nc.tensor.matmul(
                                py[:], w01[:, dmc, jc * P:(jc + 1) * P],
                                ht[:, dmc, :],
                                start=(dmc == 0), stop=(dmc == NDM - 1))
                        # GA = relu(g2*u - 0.1*g2) = g2 * relu(u - 0.1)
                        ga = tmp.tile([P, T], dt.float32, tag="tmp")
                        nc.scalar.activation(ga[:], u[:, jc, :], Act.Relu,
                                             bias=gb2[:, jc:jc + 1],
                                             scale=g2[:, jc:jc + 1])
                        # s2 = GA - Y'  (written back into the PSUM tile)
                        nc.vector.scalar_tensor_tensor(
                            py[:], py[:], -1.0, ga[:],
                            op0=Alu.mult, op1=Alu.add)
                        # s3 = 0.9*u + s2
                        s3 = tmp.tile([P, T], dt.float32, tag="tmp")
                        nc.vector.scalar_tensor_tensor(
                            s3[:], u[:, jc, :], 0.9, py[:],
                            op0=Alu.mult, op1=Alu.add)
                        # u' = s3 + B'
                        nc.vector.tensor_tensor(
                            u[:, jc, :], s3[:], bp[:, jc, :], op=Alu.add)
                        # next a = relu(u' - 0.1)
                        nc.scalar.activation(a[:, jc, :], u[:, jc, :],
                                             Act.Relu, bias=nlam[:, 0:1])

                # ---- out = a @ W^T, natural [tok, dm] layout ----
                for sub in range(T // P):
                    ob = oio.tile([P, DM], dt.float32, tag="ob")
                    for nh in range(2):
                        po = psum.tile([P, 512], dt.float32, tag="mm")
                        for kc in range(NDL):
                            nc.tensor.matmul(
                                po[:], a[:, kc, sub * P:(sub + 1) * P],
                                wt[:, kc, nh * 512:(nh + 1) * 512],
                                start=(kc == 0), stop=(kc == NDL - 1))
                        nc.scalar.copy(ob[:, nh * 512:(nh + 1) * 512], po[:])
                    row = (blk * (T // P) + sub) * P
                    nc.sync.dma_start(out_d[row:row + P, :], ob[:])

    nc.compile()
    return nc


def _get_nc():
    if "nc" not in _CACHE:
        _CACHE["nc"] = _build_nc()
    return _CACHE["nc"]


def _prep_shared(W):
    W = np.asarray(W, np.float32)
    wt = np.ascontiguousarray(
        W.T.reshape(NDL, P, DM).transpose(1, 0, 2)).astype(BF16)
    w01 = np.ascontiguousarray(
        (0.1 * W).reshape(NDM, P, DL).transpose(1, 0, 2)).astype(BF16)
    g = 0.1 * (W.astype(np.float64) ** 2).sum(0).astype(np.float64)
    g2 = np.ascontiguousarray(g.reshape(NDL, P).T).astype(np.float32)
    gb2 = (-0.1 * g2).astype(np.float32)
    return wt, w01, g2, gb2


def kernel(x, W):
    import os

    from concourse.bass_utils import run_bass_kernel_spmd

    if not TRACE:
        # the NTFF-profile path needs antenv.axon_hooks, absent here
        os.environ.setdefault("BASS_NEVER_TRACE", "1")
    x = np.asarray(x)
    orig_shape = x.shape
    xf = x.reshape(-1, DM).astype(np.float32)
    wt, w01, g2, gb2 = _prep_shared(W)

    in_maps = []
    for c in range(NCORES):
        xs = xf[c * TOK_CORE:(c + 1) * TOK_CORE]          # [1024, 1024]
        xt = np.ascontiguousarray(
            xs.reshape(NBLK, T, NDM, P).transpose(0, 3, 2, 1)).astype(BF16)
        in_maps.append({"xt": xt, "wt": wt, "w01": w01,
                        "g2": g2, "gb2": gb2})

    nc = _get_nc()
    res = run_bass_kernel_spmd(nc, in_maps, core_ids=list(range(NCORES)),
                               trace=TRACE)
    global LAST_RESULT
    LAST_RESULT = res
    out = np.concatenate([res.results[c]["out"] for c in range(NCORES)], axis=0)
    return out.reshape(orig_shape).astype(np.float32)
